# revision 14
# baseline (speedup 1.0000x reference)
"""Trainium2 Bass kernel: 6-layer dense transformer (B=2,S=1024,D=1024,H=16,DFF=4096,V=32000).

Sharding: token-data-parallel over 8 cores. Cores 0-3 handle batch 0, cores 4-7
batch 1. Within a 4-core group, the 1024 tokens (8 chunks of 128) are striped:
group-rank r owns chunks r and 7-r (balances causal attention work). Weights are
replicated (streamed from HBM in bf16); K/V are all-gathered per layer within
each group. The lm_head runs fully per core over its 256 tokens; the host
concatenates the 8 output shards. Matmuls run in bf16 with f32 PSUM accumulation;
the residual stream and layernorm statistics stay f32.
"""
import sys
import math

import numpy as np

try:
    import concourse.bass as bass
except ImportError:
    sys.path.insert(0, "/opt/trn_rl_repo")
    import concourse.bass as bass

import ml_dtypes
import concourse.mybir as mybir
import concourse.tile as tile
from concourse import bacc
from concourse.bass_utils import run_bass_kernel_spmd
from concourse.masks import make_identity

BF16 = ml_dtypes.bfloat16
FP32 = mybir.dt.float32
BF = mybir.dt.bfloat16
P = 128

# model dims
B, S = 2, 1024
D, H, DFF, L, V = 1024, 16, 4096, 6, 32000
HD = D // H  # 64
EPS = 1e-5
G = 4                 # cores per batch group
NCHUNK = S // P       # 8 global chunks per batch
ND = D // P           # 8
NF = DFF // P         # 32
NT = 2                # token slots per core (2 x 128 = 256 tokens)
VA = 65               # V head block incl. ones column
KELEM = D * 256                    # K region elems per rank (feature-major, 256 local tokens)
VELEM = 256 * (H * VA)             # V region elems per rank (token-major, augmented)
KVSZ = KELEM + VELEM
MASK_NEG = -50000.0
SCALE = 1.0 / math.sqrt(HD)


def chunk_owner(g):
    """global chunk -> (group rank, slot)"""
    return (g, 0) if g < G else (7 - g, 1)


def build_graph(nc, L_act, V_act, use_ab, use_amid, use_aout):
    """Build the SPMD program.

    use_ab:   per-partition biases usable in feature-major evacs (bq,bk,b1)
    use_amid: broadcast-tile adds for token-major evacs (bv, b2, bo)
    use_aout: layernorm affine (ln w/b broadcast tiles)
    """
    ids_d = nc.dram_tensor("ids", [NT, P], mybir.dt.int32, kind="ExternalInput")
    embed_d = nc.dram_tensor("embed", [V, D], FP32, kind="ExternalInput")
    base_lo_d = nc.dram_tensor("base_lo", [G, P, 256], FP32, kind="ExternalInput")
    base_hi_d = nc.dram_tensor("base_hi", [G, P, P], FP32, kind="ExternalInput")
    wq_d = nc.dram_tensor("wqT", [L_act, D, D], BF, kind="ExternalInput")
    wk_d = nc.dram_tensor("wkT", [L_act, D, D], BF, kind="ExternalInput")
    wv_d = nc.dram_tensor("wvT", [L_act, D, D], BF, kind="ExternalInput")
    wo_d = nc.dram_tensor("woT", [L_act, D, D], BF, kind="ExternalInput")
    w1_d = nc.dram_tensor("w1T", [L_act, D, DFF], BF, kind="ExternalInput")
    w2_d = nc.dram_tensor("w2T", [L_act, DFF, D], BF, kind="ExternalInput")
    wh_d = nc.dram_tensor("wheadT", [D, V_act], BF, kind="ExternalInput")
    if use_ab:
        bq_d = nc.dram_tensor("bq", [L_act, D], FP32, kind="ExternalInput")
        bk_d = nc.dram_tensor("bk", [L_act, D], FP32, kind="ExternalInput")
        b1_d = nc.dram_tensor("b1", [L_act, DFF], FP32, kind="ExternalInput")
    if use_amid:
        bv_d = nc.dram_tensor("bv_bc", [L_act, P, D], FP32, kind="ExternalInput")
        bo_d = nc.dram_tensor("bo_bc", [L_act, P, D], FP32, kind="ExternalInput")
        b2_d = nc.dram_tensor("b2_bc", [L_act, P, D], FP32, kind="ExternalInput")
    if use_aout:
        ln1w_d = nc.dram_tensor("ln1w_bc", [L_act, P, D], FP32, kind="ExternalInput")
        ln1b_d = nc.dram_tensor("ln1b_bc", [L_act, P, D], FP32, kind="ExternalInput")
        ln2w_d = nc.dram_tensor("ln2w_bc", [L_act, P, D], FP32, kind="ExternalInput")
        ln2b_d = nc.dram_tensor("ln2b_bc", [L_act, P, D], FP32, kind="ExternalInput")
        lnfw_d = nc.dram_tensor("lnfw_bc", [P, D], FP32, kind="ExternalInput")
        lnfb_d = nc.dram_tensor("lnfb_bc", [P, D], FP32, kind="ExternalInput")
    out_d = nc.dram_tensor("out", [NT, P, V_act], FP32, kind="ExternalOutput")
    import os as _os
    BISECT = _os.environ.get("KBISECT", "")
    if BISECT:
        dbg_d = nc.dram_tensor("dbg", [NT, P, D], FP32, kind="ExternalOutput")

    class _Halt(Exception):
        pass

    with tile.TileContext(nc) as tc:
        with tc.tile_pool(name="const", bufs=1) as cp, \
             tc.tile_pool(name="sb", bufs=2) as sb, \
             tc.tile_pool(name="ps", bufs=8, space="PSUM") as psp, \
             tc.tile_pool(name="dram", bufs=2, space="DRAM") as dp:

            def psum(h=P, w=512, dt=FP32):
                return psp.tile([h, w], dt, name="ps", tag="ps", bufs=8)

            def wtile():
                return sb.tile([P, D], BF, name="wk_t", tag="wk", bufs=12)

            ident_f = cp.tile([P, P], FP32, name="ident_f")
            make_identity(nc, ident_f[:])
            ident_b = cp.tile([P, P], BF, name="ident_b")
            make_identity(nc, ident_b[:])
            eps_t = cp.tile([P, 1], FP32, name="eps_t")
            nc.gpsimd.memset(eps_t[:], EPS)

            # alibi/causal base tiles, resident
            base_lo = []
            base_hi = []
            for g in range(G):
                t = cp.tile([P, 256], FP32, name=f"blo{g}")
                nc.sync.dma_start(out=t[:], in_=base_lo_d[g])
                base_lo.append(t)
                t = cp.tile([P, P], FP32, name=f"bhi{g}")
                nc.sync.dma_start(out=t[:], in_=base_hi_d[g])
                base_hi.append(t)

            if use_ab:
                bq_sb = cp.tile([P, ND * L_act], FP32, name="bq_sb")
                bk_sb = cp.tile([P, ND * L_act], FP32, name="bk_sb")
                b1_sb = cp.tile([P, NF * L_act], FP32, name="b1_sb")
                for l in range(L_act):
                    nc.sync.dma_start(
                        out=bq_sb[:, l * ND:(l + 1) * ND],
                        in_=bq_d[l].rearrange("(m p) -> p m", p=P))
                    nc.sync.dma_start(
                        out=bk_sb[:, l * ND:(l + 1) * ND],
                        in_=bk_d[l].rearrange("(m p) -> p m", p=P))
                    nc.sync.dma_start(
                        out=b1_sb[:, l * NF:(l + 1) * NF],
                        in_=b1_d[l].rearrange("(m p) -> p m", p=P))

            # ---- embedding gather + scale
            x_sb = []   # residual stream, token-major f32, per slot
            for s in range(NT):
                ids_sb = sb.tile([P, 1], mybir.dt.int32, name="ids_sb", tag="ids", bufs=2)
                nc.sync.dma_start(out=ids_sb[:], in_=ids_d[s, :, None])
                gth = sb.tile([P, D], FP32, name="gth", tag="gth", bufs=2)
                nc.gpsimd.indirect_dma_start(
                    out=gth[:], out_offset=None,
                    in_=embed_d[:, :],
                    in_offset=bass.IndirectOffsetOnAxis(ap=ids_sb[:, :1], axis=0),
                )
                xt = sb.tile([P, D], FP32, name="x_init", tag="xres", bufs=6)
                nc.scalar.mul(xt[:], gth[:], math.sqrt(D))
                x_sb.append(xt)

            def transpose_cast(src, f32src=True):
                """src: 2 token-major tiles (P, D) -> ND feature-major bf16 tiles (P, 256)."""
                outs = []
                ident = ident_f if f32src else ident_b
                for f in range(ND):
                    dst = sb.tile([P, 256], BF, name="xTt", tag="xT", bufs=18)
                    for s in range(NT):
                        tp = psum(P, P, dt=FP32 if f32src else BF)
                        nc.tensor.transpose(out=tp[:], in_=src[s][:, f * P:(f + 1) * P],
                                            identity=ident[:])
                        nc.vector.tensor_copy(out=dst[:, s * P:(s + 1) * P], in_=tp[:])
                    outs.append(dst)
                return outs

            def layer_norm(xs, l_tag, w_bc=None, b_bc=None):
                """xs: 2 tiles (P,D) f32 -> 2 new normalized tiles (P,D) f32."""
                outs = []
                for s in range(NT):
                    stats = sb.tile([P, 2, 6], FP32, name="lnst", tag="lnst", bufs=4)
                    for c in range(2):
                        nc.vector.bn_stats(out=stats[:, c, :], in_=xs[s][:, c * 512:(c + 1) * 512])
                    mv = sb.tile([P, 2], FP32, name="lnmv", tag="lnmv", bufs=4)
                    nc.vector.bn_aggr(out=mv[:], in_=stats[:])
                    # rstd = exp(-0.5*ln(var+eps))
                    lnv = sb.tile([P, 1], FP32, name="lnv", tag="lnv", bufs=4)
                    nc.scalar.activation(out=lnv[:], in_=mv[:, 1:2],
                                         func=mybir.ActivationFunctionType.Ln,
                                         bias=eps_t[:, 0:1])
                    rstd = sb.tile([P, 1], FP32, name="rstd", tag="rstd", bufs=4)
                    nc.scalar.activation(out=rstd[:], in_=lnv[:],
                                         func=mybir.ActivationFunctionType.Exp, scale=-0.5)
                    nmr = sb.tile([P, 1], FP32, name="nmr", tag="nmr", bufs=4)
                    nc.vector.tensor_scalar(out=nmr[:], in0=mv[:, 0:1], scalar1=rstd[:, 0:1],
                                            scalar2=-1.0, op0=mybir.AluOpType.mult,
                                            op1=mybir.AluOpType.mult)
                    xo = sb.tile([P, D], FP32, name="x_" + l_tag, tag="xres", bufs=6)
                    nc.vector.tensor_scalar(out=xo[:], in0=xs[s][:], scalar1=rstd[:, 0:1],
                                            scalar2=nmr[:, 0:1], op0=mybir.AluOpType.mult,
                                            op1=mybir.AluOpType.add)
                    if w_bc is not None:
                        nc.vector.tensor_mul(out=xo[:], in0=xo[:], in1=w_bc[:])
                    if b_bc is not None:
                        nc.vector.tensor_add(out=xo[:], in0=xo[:], in1=b_bc[:])
                    outs.append(xo)
                return outs

            def load_bcast(dram_ap, tag):
                t = sb.tile([P, D], FP32, name=tag, tag=tag, bufs=2)
                nc.sync.dma_start(out=t[:], in_=dram_ap)
                return t

            def dump_dbg(tiles, cast=False):
                for _s in range(NT):
                    if cast:
                        t = sb.tile([P, D], FP32, name="dbgt", tag="xres", bufs=6)
                        nc.vector.tensor_copy(out=t[:], in_=tiles[_s][:])
                    else:
                        t = tiles[_s]
                    nc.sync.dma_start(out=dbg_d[_s], in_=t[:])

            if BISECT == "emb":
                dump_dbg(x_sb)
                return nc
            xT = transpose_cast(x_sb, f32src=True)

            for l in range(L_act):
                # ---------- K projection (feature-major), M-outer ----------
                wk_sb = [wtile() for _ in range(ND)]
                for k in range(ND):
                    nc.sync.dma_start(out=wk_sb[k][:], in_=wk_d[l, k * P:(k + 1) * P, :])
                kv_in = dp.tile([KVSZ], BF, name="kv_in", tag="kv_in", bufs=2)
                for m in range(ND):
                    ps = psum(P, 256)
                    for k in range(ND):
                        nc.tensor.matmul(out=ps[:, :], lhsT=wk_sb[k][:, m * P:(m + 1) * P],
                                         rhs=xT[k][:], start=(k == 0), stop=(k == ND - 1))
                    kt = sb.tile([P, 256], BF, name="ktl", tag="ktl", bufs=4)
                    if use_ab:
                        nc.vector.tensor_scalar_add(kt[:], ps[:], bk_sb[:, l * ND + m: l * ND + m + 1])
                    else:
                        nc.scalar.copy(kt[:], ps[:])
                    nc.sync.dma_start(
                        out=kv_in[m * KELEM // ND:(m + 1) * KELEM // ND].rearrange(
                            "(p j) -> p j", p=P),
                        in_=kt[:])
                # ---------- V projection (token-major, augmented), k-outer ----------
                bvt = load_bcast(bv_d[l], "bv") if use_amid else None
                v_ps = [[psum(P, 512) for _ in range(2)] for _ in range(NT)]
                for k in range(ND):
                    wv_t = wtile()
                    nc.sync.dma_start(out=wv_t[:], in_=wv_d[l, k * P:(k + 1) * P, :])
                    for s in range(NT):
                        for n in range(2):
                            nc.tensor.matmul(out=v_ps[s][n][:, :],
                                             lhsT=xT[k][:, s * P:(s + 1) * P],
                                             rhs=wv_t[:, n * 512:(n + 1) * 512],
                                             start=(k == 0), stop=(k == ND - 1))
                for s in range(NT):
                    vl = sb.tile([P, H * VA], BF, name="vloc", tag="vloc", bufs=3)
                    for n in range(2):
                        ps = v_ps[s][n]
                        if use_amid:
                            nc.vector.tensor_add(out=ps[:], in0=ps[:],
                                                 in1=bvt[:, n * 512:(n + 1) * 512])
                        nc.vector.tensor_copy(
                            out=vl[:].rearrange("p (h c) -> p h c", h=H)[:, n * 8:(n + 1) * 8, 0:64],
                            in_=ps[:].rearrange("p (h c) -> p h c", h=8))
                    nc.gpsimd.memset(vl[:].rearrange("p (h c) -> p h c", h=H)[:, :, 64:65], 1.0)
                    nc.sync.dma_start(
                        out=kv_in[KELEM + s * (VELEM // NT): KELEM + (s + 1) * (VELEM // NT)]
                        .rearrange("(p c) -> p c", p=P),
                        in_=vl[:])
                # ---------- AllGather K/V ----------
                kv_out = dp.tile([G * KVSZ], BF, name="kv_out", tag="kv_out", bufs=2)
                nc.gpsimd.collective_compute(
                    "AllGather", mybir.AluOpType.bypass,
                    replica_groups=[[0, 1, 2, 3], [4, 5, 6, 7]],
                    ins=[kv_in[:].opt()], outs=[kv_out[:].opt()],
                )
                if BISECT == "kv":
                    dump_dbg(x_sb)
                    return nc
                # ---------- Q projection (feature-major), M-outer ----------
                wq_sb = [wtile() for _ in range(ND)]
                for k in range(ND):
                    nc.sync.dma_start(out=wq_sb[k][:], in_=wq_d[l, k * P:(k + 1) * P, :])
                qt = []
                for m in range(ND):
                    ps = psum(P, 256)
                    for k in range(ND):
                        nc.tensor.matmul(out=ps[:, :], lhsT=wq_sb[k][:, m * P:(m + 1) * P],
                                         rhs=xT[k][:], start=(k == 0), stop=(k == ND - 1))
                    q = sb.tile([P, 256], BF, name="qt", tag="qt", bufs=10)
                    if use_ab:
                        nc.vector.tensor_scalar_add(q[:], ps[:], bq_sb[:, l * ND + m: l * ND + m + 1])
                    else:
                        nc.scalar.copy(q[:], ps[:])
                    qt.append(q)
                # ---------- read back gathered K/V ----------
                ksb = []
                vsb = []
                for g in range(NCHUNK):
                    r, s = chunk_owner(g)
                    ktg = sb.tile([P, D], BF, name="ksb", tag="ksb", bufs=9)
                    kbase = r * KVSZ
                    nc.sync.dma_start(
                        out=ktg[:].rearrange("p (t j) -> p t j", t=ND),
                        in_=kv_out[kbase:kbase + KELEM]
                        .rearrange("(t p j) -> p t j", t=ND, p=P)[:, :, s * P:(s + 1) * P])
                    ksb.append(ktg)
                    vtg = sb.tile([P, H * VA], BF, name="vsb", tag="vsb", bufs=9)
                    vb_off = kbase + KELEM + s * (VELEM // NT)
                    nc.sync.dma_start(
                        out=vtg[:],
                        in_=kv_out[vb_off:vb_off + VELEM // NT].rearrange("(p c) -> p c", p=P))
                    vsb.append(vtg)
                if BISECT == "qread":
                    dump_dbg(x_sb)
                    return nc
                # ---------- attention ----------
                attn_sb = [sb.tile([P, D], BF, name="attnsb", tag="attnsb", bufs=3)
                           for _ in range(NT)]
                for h in range(H):
                    po = (h % 2) * 64
                    fb = h // 2
                    slope = 2.0 ** (-8.0 * (h + 1) / H)
                    att_ps = [psum(P, VA) for _ in range(NT)]
                    for g in range(NCHUNK):
                        lo = g < G
                        w = 256 if lo else P
                        ps_sc = psum(P, w)
                        nc.tensor.matmul(
                            out=ps_sc[:, :],
                            lhsT=ksb[g][po:po + 64, fb * P:(fb + 1) * P],
                            rhs=qt[fb][po:po + 64, 0:w] if lo else qt[fb][po:po + 64, P:256],
                            start=True, stop=True)
                        bt = base_lo[g] if lo else base_hi[g - G]
                        nc.vector.scalar_tensor_tensor(
                            out=ps_sc[:], in0=ps_sc[:], scalar=SCALE / slope,
                            in1=bt[:, 0:w], op0=mybir.AluOpType.mult,
                            op1=mybir.AluOpType.add)
                        probs = sb.tile([P, w], BF, name="probs", tag="probs", bufs=6)
                        nc.scalar.activation(out=probs[:], in_=ps_sc[:],
                                             func=mybir.ActivationFunctionType.Exp,
                                             scale=slope)
                        if lo:
                            for s in range(NT):
                                nc.tensor.matmul(
                                    out=att_ps[s][:, :],
                                    lhsT=probs[:, s * P:(s + 1) * P],
                                    rhs=vsb[g][:, h * VA:(h + 1) * VA],
                                    start=(g == 0), stop=(g == G - 1 and s == 0))
                        else:
                            nc.tensor.matmul(
                                out=att_ps[1][:, :],
                                lhsT=probs[:, :],
                                rhs=vsb[g][:, h * VA:(h + 1) * VA],
                                start=False, stop=(g == NCHUNK - 1))
                    for s in range(NT):
                        rec = sb.tile([P, 1], FP32, name="rec", tag="rec", bufs=6)
                        nc.vector.reciprocal(rec[:], att_ps[s][:, 64:65])
                        nc.vector.tensor_scalar_mul(
                            attn_sb[s][:, h * 64:(h + 1) * 64],
                            att_ps[s][:, 0:64], rec[:, 0:1])
                if BISECT == "attn":
                    dump_dbg(attn_sb, cast=True)
                    return nc
                # ---------- attn transpose ----------
                attnT = []
                for f in range(ND):
                    dst = sb.tile([P, 256], BF, name="attnT", tag="attnT", bufs=9)
                    for s in range(NT):
                        tp = psum(P, P, dt=BF)
                        nc.tensor.transpose(out=tp[:], in_=attn_sb[s][:, f * P:(f + 1) * P],
                                            identity=ident_b[:])
                        nc.vector.tensor_copy(out=dst[:, s * P:(s + 1) * P], in_=tp[:])
                    attnT.append(dst)
                # ---------- out-proj + residual, k-outer ----------
                bot = load_bcast(bo_d[l], "bo") if use_amid else None
                o_ps = [[psum(P, 512) for _ in range(2)] for _ in range(NT)]
                for k in range(ND):
                    wo_t = wtile()
                    nc.sync.dma_start(out=wo_t[:], in_=wo_d[l, k * P:(k + 1) * P, :])
                    for s in range(NT):
                        for n in range(2):
                            nc.tensor.matmul(out=o_ps[s][n][:, :],
                                             lhsT=attnT[k][:, s * P:(s + 1) * P],
                                             rhs=wo_t[:, n * 512:(n + 1) * 512],
                                             start=(k == 0), stop=(k == ND - 1))
                xs1 = []
                for s in range(NT):
                    xo = sb.tile([P, D], FP32, name="xs1", tag="xres", bufs=6)
                    for n in range(2):
                        nc.vector.tensor_add(out=xo[:, n * 512:(n + 1) * 512],
                                             in0=o_ps[s][n][:], in1=x_sb[s][:, n * 512:(n + 1) * 512])
                        if use_amid:
                            nc.vector.tensor_add(out=xo[:, n * 512:(n + 1) * 512],
                                                 in0=xo[:, n * 512:(n + 1) * 512],
                                                 in1=bot[:, n * 512:(n + 1) * 512])
                    xs1.append(xo)
                # ---------- LN1 ----------
                if use_aout:
                    w_bc = load_bcast(ln1w_d[l], "lnw")
                    b_bc = load_bcast(ln1b_d[l], "lnb")
                else:
                    w_bc = b_bc = None
                x1 = layer_norm(xs1, "ln1", w_bc, b_bc)
                x1T = transpose_cast(x1, f32src=True)
                if BISECT == "ln1":
                    dump_dbg(x1)
                    return nc
                # ---------- FFN1: mb-blocks, M-outer within block ----------
                h1T = []
                for mb in range(4):
                    w1_sb = [wtile() for _ in range(ND)]
                    for k in range(ND):
                        nc.sync.dma_start(
                            out=w1_sb[k][:],
                            in_=w1_d[l, k * P:(k + 1) * P, mb * D:(mb + 1) * D])
                    for mi in range(ND):
                        m = mb * ND + mi
                        ps = psum(P, 256)
                        for k in range(ND):
                            nc.tensor.matmul(out=ps[:, :],
                                             lhsT=w1_sb[k][:, mi * P:(mi + 1) * P],
                                             rhs=x1T[k][:], start=(k == 0), stop=(k == ND - 1))
                        ht = sb.tile([P, 256], BF, name="h1T", tag="h1T", bufs=33)
                        if use_ab:
                            nc.vector.tensor_scalar(
                                out=ht[:], in0=ps[:], scalar1=b1_sb[:, l * NF + m: l * NF + m + 1],
                                scalar2=0.0, op0=mybir.AluOpType.add, op1=mybir.AluOpType.max)
                        else:
                            nc.scalar.activation(out=ht[:], in_=ps[:],
                                                 func=mybir.ActivationFunctionType.Relu)
                        h1T.append(ht)
                # ---------- FFN2 + residual, k-outer ----------
                b2t = load_bcast(b2_d[l], "b2") if use_amid else None
                f_ps = [[psum(P, 512) for _ in range(2)] for _ in range(NT)]
                for k in range(NF):
                    w2_t = wtile()
                    nc.sync.dma_start(out=w2_t[:], in_=w2_d[l, k * P:(k + 1) * P, :])
                    for s in range(NT):
                        for n in range(2):
                            nc.tensor.matmul(out=f_ps[s][n][:, :],
                                             lhsT=h1T[k][:, s * P:(s + 1) * P],
                                             rhs=w2_t[:, n * 512:(n + 1) * 512],
                                             start=(k == 0), stop=(k == NF - 1))
                xs2 = []
                for s in range(NT):
                    xo = sb.tile([P, D], FP32, name="xs2", tag="xres", bufs=6)
                    for n in range(2):
                        nc.vector.tensor_add(out=xo[:, n * 512:(n + 1) * 512],
                                             in0=f_ps[s][n][:], in1=x1[s][:, n * 512:(n + 1) * 512])
                        if use_amid:
                            nc.vector.tensor_add(out=xo[:, n * 512:(n + 1) * 512],
                                                 in0=xo[:, n * 512:(n + 1) * 512],
                                                 in1=b2t[:, n * 512:(n + 1) * 512])
                    xs2.append(xo)
                # ---------- LN2 ----------
                if use_aout:
                    w_bc = load_bcast(ln2w_d[l], "lnw")
                    b_bc = load_bcast(ln2b_d[l], "lnb")
                else:
                    w_bc = b_bc = None
                x_sb = layer_norm(xs2, "ln2", w_bc, b_bc)
                xT = transpose_cast(x_sb, f32src=True)
                if BISECT == "ffn":
                    dump_dbg(x_sb)
                    return nc

            # ---------- final LN + head ----------
            if use_aout:
                w_bc = load_bcast(lnfw_d[:], "lnw")
                b_bc = load_bcast(lnfb_d[:], "lnb")
            else:
                w_bc = b_bc = None
            xf = layer_norm(x_sb, "lnf", w_bc, b_bc)
            xfT = transpose_cast(xf, f32src=True)
            # head: blocks of VBLK vocab columns, k-outer, 8 psums per block
            if V_act % 2000 == 0:
                VBLK, NIN = 2000, 4          # inner chunks of 500
            elif V_act % 2048 == 0:
                VBLK, NIN = 2048, 4          # inner chunks of 512
            else:
                raise ValueError(f"unsupported V_act {V_act}")
            CH = VBLK // NIN
            for vb in range(V_act // VBLK):
                h_ps = [[psum(P, CH) for _ in range(NIN)] for _ in range(NT)]
                for k in range(ND):
                    wh_t = sb.tile([P, VBLK], BF, name="wh_t", tag="whead", bufs=3)
                    nc.sync.dma_start(out=wh_t[:],
                                      in_=wh_d[k * P:(k + 1) * P, vb * VBLK:(vb + 1) * VBLK])
                    for s in range(NT):
                        for n in range(NIN):
                            nc.tensor.matmul(out=h_ps[s][n][:, :],
                                             lhsT=xfT[k][:, s * P:(s + 1) * P],
                                             rhs=wh_t[:, n * CH:(n + 1) * CH],
                                             start=(k == 0), stop=(k == ND - 1))
                for s in range(NT):
                    for n in range(NIN):
                        ot = sb.tile([P, CH], FP32, name="outsb", tag="outsb", bufs=4)
                        nc.scalar.copy(ot[:], h_ps[s][n][:])
                        nc.sync.dma_start(
                            out=out_d[s, :, vb * VBLK + n * CH: vb * VBLK + (n + 1) * CH],
                            in_=ot[:])
    return nc


def _prep_shared(inputs, L_act, V_act):
    """Host-side weight prep (transpose + bf16 cast), shared across cores."""
    sh = {}
    sh["embed"] = np.ascontiguousarray(np.asarray(inputs["embed"], np.float32))
    for nm, key in (("wqT", "Wq"), ("wkT", "Wk"), ("wvT", "Wv"), ("woT", "Wo")):
        w = np.asarray(inputs[key], np.float32)[:L_act]
        sh[nm] = np.ascontiguousarray(w.transpose(0, 2, 1)).astype(BF16)
    sh["w1T"] = np.ascontiguousarray(
        np.asarray(inputs["W1"], np.float32)[:L_act].transpose(0, 2, 1)).astype(BF16)
    sh["w2T"] = np.ascontiguousarray(
        np.asarray(inputs["W2"], np.float32)[:L_act].transpose(0, 2, 1)).astype(BF16)
    sh["wheadT"] = np.ascontiguousarray(
        np.asarray(inputs["Whead"], np.float32)[:V_act].T).astype(BF16)
    return sh


def _base_tiles(r):
    """Alibi/causal base tiles for group-rank r (chunks r and 7-r).

    Layout matches the on-chip scoresT tiles: partition = key position within
    sk-chunk g, free = query position (slot 0 columns 0-127, slot 1 columns
    128-255 for the merged lo tiles; slot-1-only for the hi tiles).
    """
    chunks = (r, 7 - r)
    base_lo = np.empty((G, P, 256), np.float32)
    base_hi = np.empty((G, P, P), np.float32)
    for g in range(NCHUNK):
        skg = g * P + np.arange(P, dtype=np.float32)
        for si, ch in enumerate(chunks):
            tq = ch * P + np.arange(P, dtype=np.float32)
            val = skg[:, None] - tq[None, :]          # -(tq - sk) <= 0 in causal region
            val = np.where(skg[:, None] <= tq[None, :], val, np.float32(MASK_NEG))
            if g < G:
                base_lo[g, :, si * P:(si + 1) * P] = val
            elif si == 1:
                base_hi[g - G] = val
    return base_lo, base_hi


_GRAPH_CACHE = {}


def _get_graph(L_act, V_act, use_ab, use_amid, use_aout):
    key = (L_act, V_act, use_ab, use_amid, use_aout)
    if key not in _GRAPH_CACHE:
        nc = bacc.Bacc("TRN2", target_bir_lowering=False, debug=False, num_devices=8)
        build_graph(nc, L_act, V_act, use_ab, use_amid, use_aout)
        nc.compile()
        _GRAPH_CACHE[key] = nc
    return _GRAPH_CACHE[key]


def _flags(inputs, L_act):
    nz = lambda *names: any(np.any(np.asarray(inputs[n])[:L_act] != 0) for n in names)
    use_ab = nz("bq", "bk", "b1")
    use_amid = nz("bv", "bo", "b2")
    ln_w_nontriv = any(np.any(np.asarray(inputs[n])[:L_act] != 1) for n in ("ln1_w", "ln2_w")) \
        or np.any(np.asarray(inputs["lnf_w"]) != 1)
    use_aout = ln_w_nontriv or nz("ln1_b", "ln2_b") or np.any(np.asarray(inputs["lnf_b"]) != 0)
    return use_ab, use_amid, use_aout


def _make_in_maps(inputs, L_act, V_act, use_ab, use_amid, use_aout):
    sh = _prep_shared(inputs, L_act, V_act)
    ids_full = np.asarray(inputs["input_ids"]).astype(np.int32)  # (B, S)

    in_maps = []
    for c in range(8):
        b, r = c // G, c % G
        chunks = (r, 7 - r)
        m = dict(sh)
        m["ids"] = np.ascontiguousarray(
            np.stack([ids_full[b, ch * P:(ch + 1) * P] for ch in chunks]))
        blo, bhi = _base_tiles(r)
        m["base_lo"], m["base_hi"] = blo, bhi
        if use_ab:
            m["bq"] = np.asarray(inputs["bq"], np.float32)[:L_act]
            m["bk"] = np.asarray(inputs["bk"], np.float32)[:L_act]
            m["b1"] = np.asarray(inputs["b1"], np.float32)[:L_act]
        if use_amid:
            for nm, key in (("bv_bc", "bv"), ("bo_bc", "bo"), ("b2_bc", "b2")):
                w = np.asarray(inputs[key], np.float32)[:L_act]
                m[nm] = np.ascontiguousarray(
                    np.broadcast_to(w[:, None, :], (L_act, P, D)))
        if use_aout:
            for nm, key in (("ln1w_bc", "ln1_w"), ("ln1b_bc", "ln1_b"),
                            ("ln2w_bc", "ln2_w"), ("ln2b_bc", "ln2_b")):
                w = np.asarray(inputs[key], np.float32)[:L_act]
                m[nm] = np.ascontiguousarray(np.broadcast_to(w[:, None, :], (L_act, P, D)))
            m["lnfw_bc"] = np.ascontiguousarray(
                np.broadcast_to(np.asarray(inputs["lnf_w"], np.float32)[None, :], (P, D)))
            m["lnfb_bc"] = np.ascontiguousarray(
                np.broadcast_to(np.asarray(inputs["lnf_b"], np.float32)[None, :], (P, D)))
        in_maps.append(m)
    return in_maps


def _assemble(results, V_act):
    out = np.empty((B, S, V_act), np.float32)
    for c in range(8):
        b, r = c // G, c % G
        o = results[c]["out"]  # (NT, P, V_act)
        for si, ch in enumerate((r, 7 - r)):
            out[b, ch * P:(ch + 1) * P, :] = o[si]
    return out


def kernel(_L_act=None, _V_act=None, _trace=False, **inputs):
    L_act = L_act if (L_act := _L_act) is not None else L
    V_act = V_act if (V_act := _V_act) is not None else V
    use_ab, use_amid, use_aout = _flags(inputs, L_act)
    nc = _get_graph(L_act, V_act, use_ab, use_amid, use_aout)
    in_maps = _make_in_maps(inputs, L_act, V_act, use_ab, use_amid, use_aout)
    res = run_bass_kernel_spmd(nc, in_maps, core_ids=list(range(8)), trace=_trace)
    kernel.last_exec_time_ns = res.exec_time_ns
    kernel.last_result = res
    return _assemble(res.results, V_act)


kernel.last_exec_time_ns = None
kernel.last_result = None
